# revision 9
# baseline (speedup 1.0000x reference)
"""TRN2 Bass kernel for nn_AttentionMP (GNN message passing attention).

Row-parallel attention across 8 NeuronCores: core c owns query rows
[c*1024, (c+1)*1024). Each core holds full k/v (computed on-device from the
replicated H^T), its slice of q, and its shard of adj^T.

Per-core layout: scores are computed TRANSPOSED, sT[j, i] (j = key index on
partitions, i = this core's query rows on the free dim), which makes
att^T directly available as the moving operand of the att@v matmul — no
on-device transposes in the hot path.

Masking: adj^T shard is shipped as fp8 (values 0/1, exact) and added into
the scores PSUM as 240*adj via an identity matmul (lhsT = 240*I fp8);
ACT then computes exp(s + 240*m - 270) = exp(s - 30) for unmasked,
exp(s - 270-ish) -> 0.0 exactly for masked (matches the reference's
-1e6 additive mask after softmax). The -30 is a global stabilizer that
cancels in normalization.

Normalization is deferred: U^T = v^T @ e (unnormalized), denominators are
accumulated on DVE and folded into the MLP via G = U@W1 + denom*b1 followed
by a per-partition multiply with 1/denom on ACT.
"""
import numpy as np
import ml_dtypes
import concourse.bass as bass
from concourse import bacc
import concourse.mybir as mybir
from concourse.tile import TileContext
from concourse.bass_utils import run_bass_kernel_spmd

N = 8192
D = 128
NC = 8
RPC = N // NC          # rows per core = 1024
JT = N // 128          # j tiles = 64
F32 = mybir.dt.float32
F32R = mybir.dt.float32r
BF16 = mybir.dt.bfloat16
FP8 = mybir.dt.float8e4
MASK_D = 240.0         # fp8e4 max finite
STAB = 30.0            # global score shift, cancels in softmax
EXP_BIAS = -(MASK_D + STAB)

_CACHED = {}


def build():
    nc = bacc.Bacc("TRN2", target_bir_lowering=False, debug=True)

    HT = nc.dram_tensor("HT", [D, N], F32R, kind="ExternalInput")
    HTq = nc.dram_tensor("HTq", [D, RPC], F32R, kind="ExternalInput")
    ADJ8 = nc.dram_tensor("ADJ8", [N, RPC], FP8, kind="ExternalInput")
    WQ = nc.dram_tensor("WQ", [D, D], F32R, kind="ExternalInput")
    WK = nc.dram_tensor("WK", [D, D], F32R, kind="ExternalInput")
    WV = nc.dram_tensor("WV", [D, D], F32R, kind="ExternalInput")
    W1 = nc.dram_tensor("W1", [D, D], F32R, kind="ExternalInput")
    W2 = nc.dram_tensor("W2", [D, D], F32R, kind="ExternalInput")
    B1 = nc.dram_tensor("B1", [1, D], F32R, kind="ExternalInput")
    B2 = nc.dram_tensor("B2", [1, D], F32R, kind="ExternalInput")
    I240 = nc.dram_tensor("I240", [D, D], FP8, kind="ExternalInput")
    ONES = nc.dram_tensor("ONES", [D, D], F32R, kind="ExternalInput")
    ONE1 = nc.dram_tensor("ONE1", [1, D], F32R, kind="ExternalInput")
    IDENT = nc.dram_tensor("IDENT", [D, D], F32, kind="ExternalInput")
    BIASC = nc.dram_tensor("BIASC", [D, 1], F32, kind="ExternalInput")
    OUT = nc.dram_tensor("OUT", [RPC, D], F32, kind="ExternalOutput")

    ADJ_BATCH = 4  # j-tiles per adj DMA (512KB transfers)
    adj_view = ADJ8.rearrange("(b k p) i -> b p k i", k=ADJ_BATCH, p=128)

    with TileContext(nc) as tc:
        with (
            tc.tile_pool(name="pers", bufs=1) as pers,
            tc.tile_pool(name="adjp", bufs=3) as adjp,
            tc.tile_pool(name="ep", bufs=3) as ep,
            tc.tile_pool(name="psA", bufs=2, space="PSUM") as psA,   # [128,1024] tiles
            tc.tile_pool(name="psB", bufs=2, space="PSUM") as psB,   # [128,128] tiles
            tc.tile_pool(name="psU", bufs=1, space="PSUM") as psU,   # U accumulator
        ):
            # ---- constant / persistent tiles
            ht = pers.tile([D, N], F32R, tag="ht")
            htq = pers.tile([D, RPC], F32R, tag="htq")
            wq = pers.tile([D, D], F32R, tag="wq")
            wk = pers.tile([D, D], F32R, tag="wk")
            wv = pers.tile([D, D], F32R, tag="wv")
            w1 = pers.tile([D, D], F32R, tag="w1")
            w2 = pers.tile([D, D], F32R, tag="w2")
            b1 = pers.tile([1, D], F32R, tag="b1")
            b2 = pers.tile([1, D], F32R, tag="b2")
            i240 = pers.tile([D, D], FP8, tag="i240")
            ones = pers.tile([D, D], F32R, tag="ones")
            one1 = pers.tile([1, D], F32R, tag="one1")
            ident = pers.tile([D, D], F32, tag="ident")
            biasc = pers.tile([D, 1], F32, tag="biasc")
            for t, src in [(ht, HT), (htq, HTq), (wq, WQ), (wk, WK), (wv, WV),
                           (w1, W1), (w2, W2), (b1, B1), (b2, B2),
                           (i240, I240), (ones, ONES), (one1, ONE1), (ident, IDENT),
                           (biasc, BIASC)]:
                nc.sync.dma_start(out=t[:], in_=src[:])

            kt = pers.tile([D, N], F32R, tag="kt")      # k^T, d on partitions
            qt = pers.tile([D, RPC], F32R, tag="qt")    # q^T slice
            vsb = pers.tile([D, N], F32R, tag="vsb")    # v natural, block jt at cols [jt*128,...)
            acc = pers.tile([D, RPC], F32, tag="acc")   # denominator partial sums
            ut = pers.tile([D, RPC], F32R, tag="ut")    # U^T
            dent = pers.tile([1, RPC], F32R, tag="dent")  # denom row
            rcol = pers.tile([D, NC], F32, tag="rcol")  # 1/denom per i-tile column
            outsb = pers.tile([D, NC * D], F32, tag="outsb")

            # ---- stage 0: kT, qT, v
            for t in range(N // 1024):
                ps = psA.tile([D, 1024], F32, tag="big")
                for h in range(2):
                    nc.tensor.matmul(ps[:, h * 512:(h + 1) * 512], lhsT=wk[:],
                                     rhs=ht[:, t * 1024 + h * 512: t * 1024 + (h + 1) * 512],
                                     start=True, stop=True)
                nc.scalar.copy(kt[:, t * 1024:(t + 1) * 1024], ps[:])
            ps = psA.tile([D, 1024], F32, tag="big")
            for h in range(2):
                nc.tensor.matmul(ps[:, h * 512:(h + 1) * 512], lhsT=wq[:],
                                 rhs=htq[:, h * 512:(h + 1) * 512], start=True, stop=True)
            nc.scalar.copy(qt[:], ps[:])
            for jt in range(JT):
                pv = psB.tile([D, D], F32, tag="small")
                col = ht[:, jt * 128:(jt + 1) * 128]
                nc.tensor.matmul(pv[:], lhsT=col, rhs=wv[:], start=True, stop=True)
                nc.vector.tensor_copy(vsb[:, jt * 128:(jt + 1) * 128], pv[:])

            # ---- stage 1: masked attention, transposed scores
            ups = psU.tile([D, RPC], F32, tag="u")
            for b in range(JT // ADJ_BATCH):
                adj_sb = adjp.tile([128, ADJ_BATCH * RPC], FP8, tag="adj")
                nc.sync.dma_start(
                    out=adj_sb[:].rearrange("p (k i) -> p k i", k=ADJ_BATCH),
                    in_=adj_view[b])
                for k in range(ADJ_BATCH):
                    jt = b * ADJ_BATCH + k
                    sps = psA.tile([D, RPC], F32, tag="big")
                    ktile = kt[:, jt * 128:(jt + 1) * 128]
                    for h in range(2):
                        cs = slice(h * 512, (h + 1) * 512)
                        nc.tensor.matmul(sps[:, cs], lhsT=ktile, rhs=qt[:, cs],
                                         start=True, stop=False)
                    for h in range(2):
                        cs = slice(h * 512, (h + 1) * 512)
                        nc.tensor.matmul(sps[:, cs], lhsT=i240[:],
                                         rhs=adj_sb[:, k * RPC + h * 512: k * RPC + (h + 1) * 512],
                                         start=False, stop=True)
                    e = ep.tile([D, RPC], F32R, tag="e")
                    nc.scalar.activation(e[:], sps[:], mybir.ActivationFunctionType.Exp,
                                         bias=biasc[:])
                    vtile = vsb[:, jt * 128:(jt + 1) * 128]
                    for h in range(2):
                        cs = slice(h * 512, (h + 1) * 512)
                        nc.tensor.matmul(ups[:, cs], lhsT=vtile, rhs=e[:, cs],
                                         start=(jt == 0), stop=(jt == JT - 1))
                    if jt == 0:
                        nc.vector.tensor_copy(acc[:], e[:])
                    else:
                        nc.vector.tensor_add(acc[:], acc[:], e[:])

            # ---- stage 2: denominators, normalization-deferred MLP
            accr = pers.tile([D, RPC], F32R, tag="accr")
            nc.vector.tensor_copy(accr[:], acc[:])
            dps = psA.tile([D, RPC], F32, tag="big")
            for h in range(2):
                cs = slice(h * 512, (h + 1) * 512)
                nc.tensor.matmul(dps[:, cs], lhsT=ones[:], rhs=accr[:, cs],
                                 start=True, stop=True)
            nc.scalar.copy(dent[:], dps[0:1, :])
            nc.scalar.copy(ut[:], ups[:])

            # 1/denom as per-partition columns: transpose dent in 128-chunks
            rps = psB.tile([D, NC], F32, tag="small")
            for it in range(NC):
                nc.tensor.transpose(rps[:, it:it + 1],
                                    dent[0:1, it * 128:(it + 1) * 128].bitcast(F32),
                                    ident[0:1, 0:1])
            dcol = pers.tile([D, NC], F32, tag="dcol")
            nc.scalar.copy(dcol[:], rps[:])
            nc.vector.reciprocal(rcol[:], dcol[:])

            for it in range(NC):
                gps = psB.tile([D, D], F32, tag="small")
                nc.tensor.matmul(gps[:], lhsT=ut[:, it * 128:(it + 1) * 128],
                                 rhs=w1[:], start=True, stop=False)
                nc.tensor.matmul(gps[:], lhsT=dent[0:1, it * 128:(it + 1) * 128],
                                 rhs=b1[:], start=False, stop=True)
                h_sb = ep.tile([D, D], F32, tag="hsb")
                nc.scalar.activation(h_sb[:], gps[:], mybir.ActivationFunctionType.Relu,
                                     scale=rcol[:, it:it + 1])
                tps = psB.tile([D, D], F32, tag="small")
                nc.tensor.transpose(tps[:], h_sb[:], ident[:])
                htr = ep.tile([D, D], F32R, tag="htr")
                nc.scalar.copy(htr[:], tps[:])
                ops = psB.tile([D, D], F32, tag="small")
                nc.tensor.matmul(ops[:], lhsT=htr[:], rhs=w2[:], start=True, stop=False)
                nc.tensor.matmul(ops[:], lhsT=one1[:], rhs=b2[:], start=False, stop=True)
                nc.scalar.activation(outsb[:, it * 128:(it + 1) * 128], ops[:],
                                     mybir.ActivationFunctionType.Relu)

            nc.sync.dma_start(out=OUT.rearrange("(t p) d -> p t d", p=128),
                              in_=outsb[:].rearrange("p (t d) -> p t d", t=NC))
    nc.finalize()
    return nc


def _prep(H, adj, Wq, Wk, Wv, W1, b1, W2, b2):
    f8 = ml_dtypes.float8_e4m3
    HT = np.ascontiguousarray(np.asarray(H, dtype=np.float32).T)
    adj = np.asarray(adj)
    base = {
        "HT": HT,
        "WQ": np.asarray(Wq, np.float32), "WK": np.asarray(Wk, np.float32),
        "W1": np.asarray(W1, np.float32), "W2": np.asarray(W2, np.float32),
        "B1": np.asarray(b1, np.float32).reshape(1, D),
        "B2": np.asarray(b2, np.float32).reshape(1, D),
        "I240": (np.eye(D, dtype=np.float32) * MASK_D).astype(f8),
        "ONES": np.ones((D, D), np.float32),
        "ONE1": np.ones((1, D), np.float32),
        "IDENT": np.eye(D, dtype=np.float32),
        "BIASC": np.full((D, 1), -(MASK_D + STAB), np.float32),
    }
    base["WV"] = np.asarray(Wv, np.float32)
    in_maps = []
    for c in range(NC):
        m = dict(base)
        m["HTq"] = np.ascontiguousarray(HT[:, c * RPC:(c + 1) * RPC])
        m["ADJ8"] = np.ascontiguousarray(
            adj[c * RPC:(c + 1) * RPC, :].T).astype(np.float32).astype(f8)
        in_maps.append(m)
    return in_maps


def kernel(H, adj, Wq, Wk, Wv, W1, b1, W2, b2):
    if "nc" not in _CACHED:
        _CACHED["nc"] = build()
    in_maps = _prep(H, adj, Wq, Wk, Wv, W1, b1, W2, b2)
    res = run_bass_kernel_spmd(_CACHED["nc"], in_maps, list(range(NC)))
    return np.concatenate([res.results[c]["OUT"] for c in range(NC)], axis=0)
